# revision 21
# baseline (speedup 1.0000x reference)
"""Soft-MoE (4 heterogeneous experts + shared expert) Trainium2 kernel.

Strategy (8 NeuronCores, data-parallel over tokens):
  - x sharded along T: each core gets Tl = 4096 tokens, in BOTH layouts
    (token-major for the dispatch contraction, D-major for everything else).
  - Phase A (per core): router logits, E = exp(logits), per-token combine,
    partial slot inputs P_l = E^T @ x_l, partial colsum/usage/z-loss stats.
  - AllReduce #1: P (4x1024) + stats row.
  - Phase B: slot_in = P/colsum; heterogeneous expert FFNs evaluated as
    GEMVs with hidden dims column-sharded across the 8 cores (uniform SPMD
    program, per-core weight slices fed as inputs); AllReduce #2 merges the
    second-layer partial sums; math expert layer 3 uses a replicated w3.
    Loss (router + z + ortho) computed on-device.
  - Phase C (per core): shared expert gelu(x@w1)@w2 with the slot-combine
    matmul accumulated into the same PSUM group, + per-D bias; output is
    written transposed (outT [D, Tl]) and re-assembled on the host.

All matmuls run in float32r (TF32-like, ~1.5e-4 rel err, full PE rate).
"""

import numpy as np

T, D, H = 32768, 1024, 2048
E_EXP, SPE = 4, 1
S = E_EXP * SPE
TAU = 1.0
Z_W, ORTHO_W = 0.01, 0.01
LN_EPS = 1e-5

NCORES = 8
TL = T // NCORES          # 4096 tokens per core
TA = 512                  # phase A token tile
NTA = TL // TA            # 8
TC = 256                  # phase C token tile
NTC = TL // TC            # 16
MH_SL = (2 * H) // NCORES  # 512  math hidden slice per core
EH_SL = H // NCORES        # 256  l/c/s hidden slice per core


def _build_program():
    import concourse.tile as tile
    from concourse import bacc, mybir
    from contextlib import ExitStack

    F32 = mybir.dt.float32
    F32R = mybir.dt.float32r
    AF = mybir.ActivationFunctionType
    OP = mybir.AluOpType
    AX = mybir.AxisListType

    def ts(i, n):
        return slice(i * n, (i + 1) * n)

    nc = bacc.Bacc("TRN2", target_bir_lowering=False, debug=False,
                   num_devices=NCORES)

    # ---- DRAM I/O ----
    xt_d = nc.dram_tensor("xt", [D, TL], F32, kind="ExternalInput")
    xm_d = nc.dram_tensor("xm", [TL, D], F32, kind="ExternalInput")
    rwt_d = nc.dram_tensor("rwt", [D, S], F32, kind="ExternalInput")
    rw_d = nc.dram_tensor("rw", [S, D], F32, kind="ExternalInput")
    sbias_d = nc.dram_tensor("slot_bias", [S, 1], F32, kind="ExternalInput")
    id_d = nc.dram_tensor("ident", [128, 128], F32, kind="ExternalInput")
    w1t_d = nc.dram_tensor("w1t", [D, H], F32, kind="ExternalInput")
    w2t_d = nc.dram_tensor("w2t", [H, D], F32, kind="ExternalInput")
    shb1_d = nc.dram_tensor("shb1", [128, H // 128], F32, kind="ExternalInput")
    shb2_d = nc.dram_tensor("shb2", [128, D // 128], F32, kind="ExternalInput")
    mw1t_d = nc.dram_tensor("mw1t", [D, MH_SL], F32, kind="ExternalInput")
    mb1_d = nc.dram_tensor("mb1", [1, MH_SL], F32, kind="ExternalInput")
    mw2t_d = nc.dram_tensor("mw2t", [MH_SL, H], F32, kind="ExternalInput")
    mb2_d = nc.dram_tensor("mb2", [1, H], F32, kind="ExternalInput")
    mw3t_d = nc.dram_tensor("mw3t", [H, D], F32, kind="ExternalInput")
    mb3_d = nc.dram_tensor("mb3", [1, D], F32, kind="ExternalInput")
    ew1t_d, eb1_d, ew2t_d, eb2_d = {}, {}, {}, {}
    for e in ("l", "c", "s"):
        ew1t_d[e] = nc.dram_tensor(f"{e}w1t", [D, EH_SL], F32, kind="ExternalInput")
        eb1_d[e] = nc.dram_tensor(f"{e}b1", [1, EH_SL], F32, kind="ExternalInput")
        ew2t_d[e] = nc.dram_tensor(f"{e}w2t", [EH_SL, D], F32, kind="ExternalInput")
        eb2_d[e] = nc.dram_tensor(f"{e}b2", [1, D], F32, kind="ExternalInput")
    lg_d = nc.dram_tensor("lg", [1, D], F32, kind="ExternalInput")
    lbe_d = nc.dram_tensor("lbe", [1, D], F32, kind="ExternalInput")

    outT_d = nc.dram_tensor("outT", [D, TL], F32, kind="ExternalOutput")
    loss_d = nc.dram_tensor("loss", [1, 1], F32, kind="ExternalOutput")

    combT_d = nc.dram_tensor("combT_scratch", [S, TL], F32)
    ar1_in = nc.dram_tensor("ar1_in", [5, D], F32)
    ar1_out = nc.dram_tensor("ar1_out", [5, D], F32, addr_space="Shared")
    ar2_in = nc.dram_tensor("ar2_in", [1, H + 3 * D], F32)
    ar2_out = nc.dram_tensor("ar2_out", [1, H + 3 * D], F32,
                             addr_space="Shared")
    RG = [list(range(NCORES))]

    with tile.TileContext(nc) as tc, ExitStack() as octx:
        pers = octx.enter_context(tc.tile_pool(name="pers", bufs=1))

        # resident small tensors
        rwt_sb = pers.tile([128, D // 128, S], F32R)
        nc.sync.dma_start(out=rwt_sb, in_=rwt_d[:, :].rearrange(
            "(k p) s -> p k s", p=128).bitcast(F32R))
        ident = pers.tile([128, 128], F32)
        nc.sync.dma_start(out=ident, in_=id_d[:, :])
        sbias = pers.tile([S, 1], F32)
        nc.sync.dma_start(out=sbias, in_=sbias_d[:, :])
        shb1 = pers.tile([128, H // 128], F32)
        nc.sync.dma_start(out=shb1, in_=shb1_d[:, :])
        shb2 = pers.tile([128, D // 128], F32)
        nc.sync.dma_start(out=shb2, in_=shb2_d[:, :])
        cs_acc = pers.tile([S, NTA], F32)      # exp colsum partials per tile
        slot_out = pers.tile([S, D], F32R)     # final expert outputs
        ortho_ss = pers.tile([1, 1], F32)
        ones128 = nc.const_aps.tensor(1.0, (128, 1))
        ones4 = nc.const_aps.tensor(1.0, (4, 1))

        # w2t loads early: phase A is DMA-issue-bound, not BW-saturated, so
        # these 16.8MB hide under it and phase C's mm2 never waits on them
        pEw = octx.enter_context(tc.tile_pool(name="pEw", bufs=1))
        w2t_sb = pEw.tile([128, H // 128, D], F32R)

        # =====================  PHASE A  =====================
        with tc.tile_pool(name="pA", bufs=2) as pA, \
             tc.tile_pool(name="psA", bufs=1, space="PSUM") as psA:
            psum_P = psA.tile([S, D], F32)        # slot-input partials
            psum_zu = psA.tile([1, 20], F32)      # usage(16) + zsq(4)

            for t in range(NTA):
                xt_t = pA.tile([128, D // 128, TA], F32R, tag="xt")
                nc.sync.dma_start(out=xt_t, in_=xt_d[:, ts(t, TA)].rearrange(
                    "(k p) c -> p k c", p=128).bitcast(F32R))
                xm_t = pA.tile([128, TA // 128, D], F32R, tag="xm")
                nc.sync.dma_start(out=xm_t, in_=xm_d[ts(t, TA), :].rearrange(
                    "(c p) d -> p c d", p=128).bitcast(F32R))
                if t >= 1:
                    for wk in range((t - 1) * 2, min(t * 2, H // 128)):
                        nc.sync.dma_start(
                            out=w2t_sb[:, wk, :],
                            in_=w2t_d[ts(wk, 128), :].bitcast(F32R))

                # logits^T [S, TA] (slot-major)
                ps_lt = psA.tile([S, TA], F32, tag="lt")
                for k in range(D // 128):
                    nc.tensor.matmul(ps_lt, rwt_sb[:, k, :], xt_t[:, k, :],
                                     start=(k == 0), stop=(k == D // 128 - 1))
                # E^T = exp(logits/TAU + bias); accum -> colsum partial
                eT = pA.tile([S, TA], F32, tag="eT")
                nc.scalar.activation(eT, ps_lt, AF.Exp, bias=sbias,
                                     scale=1.0 / TAU,
                                     accum_out=cs_acc[:, t:t + 1])

                e_tok = pA.tile([128, TA // 128, S], F32R, tag="etok")
                rs = pA.tile([128, TA // 128], F32, tag="rs")
                stats = pA.tile([128, 20], F32, tag="stats")
                combT_sb = pA.tile([S, TA], F32R, tag="cTs")
                for cch in range(TA // 128):
                    ps_e = psA.tile([128, S], F32, tag="pe_t", bufs=2)
                    nc.tensor.transpose(ps_e, eT[:, ts(cch, 128)],
                                        ident[:S, :S])
                    nc.scalar.copy(e_tok[:, cch, :], ps_e)
                    nc.vector.tensor_reduce(rs[:, cch:cch + 1], ps_e,
                                            axis=AX.X, op=OP.add)
                    rr = pA.tile([128, 1], F32, tag="rr", bufs=3)
                    nc.vector.reciprocal(rr, rs[:, cch:cch + 1])
                    # combine chunk
                    nc.vector.tensor_scalar_mul(
                        stats[:, cch * S:(cch + 1) * S], ps_e, rr)
                    # z-term chunk: min(log(rowsum),10)^2
                    zl = pA.tile([128, 1], F32, tag="zl", bufs=3)
                    nc.scalar.activation(zl, rs[:, cch:cch + 1], AF.Ln)
                    nc.vector.tensor_scalar_min(zl, zl, 10.0)
                    nc.scalar.activation(stats[:, 16 + cch:17 + cch], zl,
                                         AF.Square)
                    # combine^T for phase C
                    ps_ct = psA.tile([S, 128], F32, tag="ct", bufs=2)
                    nc.tensor.transpose(ps_ct, stats[:, cch * S:(cch + 1) * S],
                                        ident[:128, :128])
                    nc.scalar.copy(combT_sb[:, ts(cch, 128)], ps_ct)
                nc.gpsimd.dma_start(out=combT_d[:, ts(t, TA)].bitcast(F32R),
                                    in_=combT_sb)
                # stats reduction over tokens (partition sum via PE)
                nc.tensor.matmul(psum_zu, ones128, stats, start=(t == 0),
                                 stop=(t == NTA - 1), skip_group_check=True)
                # P partial: E^T @ x
                for cch in range(TA // 128):
                    for h in range(2):
                        nc.tensor.matmul(
                            psum_P[:, ts(h, 512)], e_tok[:, cch, :],
                            xm_t[:, cch, ts(h, 512)],
                            start=(t == 0 and cch == 0),
                            stop=(t == NTA - 1 and cch == TA // 128 - 1),
                            skip_group_check=True)

            for wk in range(14, H // 128):
                nc.sync.dma_start(out=w2t_sb[:, wk, :],
                                  in_=w2t_d[ts(wk, 128), :].bitcast(F32R))

            # ---- ortho loss (router weight only) ----
            rw_sb = pA.tile([S, D], F32, tag="rw_sb", bufs=1)
            nc.sync.dma_start(out=rw_sb, in_=rw_d[:, :])
            scr = pA.tile([S, D], F32, tag="scr", bufs=1)
            nrm = pA.tile([S, 1], F32, tag="nrm", bufs=1)
            nc.scalar.activation(scr, rw_sb, AF.Square, accum_out=nrm)
            nc.scalar.activation(nrm, nrm, AF.Sqrt)
            nc.vector.tensor_scalar_add(nrm, nrm, 1e-8)
            rnrm = pA.tile([S, 1], F32, tag="rnrm", bufs=1)
            nc.vector.reciprocal(rnrm, nrm)
            wn = pA.tile([S, D], F32, tag="wn", bufs=1)
            nc.scalar.activation(wn, rw_sb, AF.Copy, scale=rnrm)
            wnT = pA.tile([128, D // 128, S], F32R, tag="wnT", bufs=1)
            for k in range(D // 128):
                ps_w = psA.tile([128, S], F32, tag="pe_t", bufs=2)
                nc.tensor.transpose(ps_w, wn[:, ts(k, 128)], ident[:S, :S])
                nc.scalar.copy(wnT[:, k, :], ps_w)
            ps_g = psA.tile([S, S], F32, tag="lt")
            for k in range(D // 128):
                nc.tensor.matmul(ps_g, wnT[:, k, :], wnT[:, k, :],
                                 start=(k == 0), stop=(k == D // 128 - 1))
            gram = pA.tile([S, S], F32, tag="gram_sb", bufs=1)
            nc.scalar.copy(gram, ps_g)
            nc.vector.tensor_sub(gram, gram, ident[:S, :S])
            osq = pA.tile([S, S], F32, tag="osq", bufs=1)
            orow = pA.tile([S, 1], F32, tag="orow", bufs=1)
            nc.scalar.activation(osq, gram, AF.Square, accum_out=orow)
            ps_o = psA.tile([1, 1], F32, tag="ct", bufs=2)
            nc.tensor.matmul(ps_o, ones4, orow, start=True, stop=True)
            nc.scalar.copy(ortho_ss, ps_o)

            # ---- pack + AllReduce #1 ----
            P_sb = pA.tile([S, D], F32, tag="P_sb", bufs=1)
            nc.scalar.copy(P_sb, psum_P)
            stats_row = pA.tile([1, D], F32, tag="stats_row", bufs=1)
            nc.vector.memset(stats_row, 0.0)
            nc.scalar.copy(stats_row[:, 0:20], psum_zu)
            colsum = pA.tile([S, 1], F32, tag="colsum", bufs=1)
            nc.vector.tensor_reduce(colsum, cs_acc, axis=AX.X, op=OP.add)
            ps_cs = psA.tile([1, S], F32, tag="ct", bufs=2)
            nc.tensor.transpose(ps_cs, colsum, ident[:S, :S])
            nc.scalar.copy(stats_row[:, 20:24], ps_cs)
            nc.gpsimd.dma_start(out=ar1_in[0:S, :], in_=P_sb)
            nc.gpsimd.dma_start(out=ar1_in[S:S + 1, :], in_=stats_row)

        nc.gpsimd.collective_compute(
            "AllReduce", mybir.AluOpType.add,
            replica_groups=RG, ins=[ar1_in[:, :]], outs=[ar1_out[:, :]])

        # =====================  PHASE B  =====================
        bctx = ExitStack()
        pBb = bctx.enter_context(tc.tile_pool(name="pBb", bufs=1))
        biases = {}
        for nm, dt_ in (("mb1", mb1_d), ("mb2", mb2_d), ("mb3", mb3_d),
                        ("lb1", eb1_d["l"]), ("lb2", eb2_d["l"]),
                        ("cb1", eb1_d["c"]), ("cb2", eb2_d["c"]),
                        ("sb1", eb1_d["s"]), ("sb2", eb2_d["s"]),
                        ("lg", lg_d), ("lbe", lbe_d)):
            t_ = pBb.tile(list(dt_.shape), F32, tag=f"bias_{nm}")
            nc.sync.dma_start(out=t_, in_=dt_[:, :])
            biases[nm] = t_
        with tc.tile_pool(name="pB", bufs=1) as pB, \
             tc.tile_pool(name="pBs", bufs=3) as pBs, \
             tc.tile_pool(name="psB", bufs=1, space="PSUM") as psB:
            P_tot = pB.tile([S, D], F32)
            nc.sync.dma_start(out=P_tot, in_=ar1_out[0:S, :])
            stats_tot = pers.tile([1, D], F32)
            nc.sync.dma_start(out=stats_tot, in_=ar1_out[S:S + 1, :])

            # slot_in = P / colsum
            ps_cs4 = psB.tile([S, 1], F32, tag="tpx", bufs=2)
            nc.tensor.transpose(ps_cs4, stats_tot[:, 20:24], ident[:1, :1])
            cs4 = pB.tile([S, 1], F32)
            nc.scalar.copy(cs4, ps_cs4)
            rc4 = pB.tile([S, 1], F32)
            nc.vector.reciprocal(rc4, cs4)
            slot_in = pB.tile([S, D], F32)
            nc.scalar.activation(slot_in, P_tot, AF.Copy, scale=rc4)
            slotT = pB.tile([128, D // 128, S], F32R)
            for k in range(D // 128):
                ps_t = psB.tile([128, S], F32, tag="tpx", bufs=2)
                nc.tensor.transpose(ps_t, slot_in[:, ts(k, 128)],
                                    ident[:S, :S])
                nc.scalar.copy(slotT[:, k, :], ps_t)

            # LayerNorm on language slot -> replace slotT[:, :, 1]
            xl_raw = pB.tile([1, D], F32)
            nc.gpsimd.dma_start(out=xl_raw, in_=P_tot[1:2, :])
            rc1 = pB.tile([1, 1], F32)
            nc.gpsimd.dma_start(out=rc1, in_=rc4[1:2, :])
            xl = pB.tile([1, D], F32)
            nc.scalar.activation(xl, xl_raw, AF.Copy, scale=rc1)
            bst = pB.tile([1, 2, 6], F32)
            for h in range(2):
                nc.vector.bn_stats(bst[:, h, :], xl[:, ts(h, 512)])
            mv = pB.tile([1, 2], F32)
            nc.vector.bn_aggr(mv, bst)
            eps_t = pB.tile([1, 1], F32)
            nc.vector.memset(eps_t, LN_EPS)
            sd = pB.tile([1, 1], F32)
            nc.scalar.activation(sd, mv[:, 1:2], AF.Sqrt, bias=eps_t)
            rstd = pB.tile([1, 1], F32)
            nc.vector.reciprocal(rstd, sd)
            xln = pB.tile([1, D], F32)
            nc.vector.tensor_scalar(xln, xl, mv[:, 0:1], rstd,
                                    op0=OP.subtract, op1=OP.mult)
            nc.vector.tensor_mul(xln, xln, biases["lg"])
            nc.vector.tensor_add(xln, xln, biases["lbe"])
            for k in range(D // 128):
                ps_t = psB.tile([128, 1], F32, tag="tpx", bufs=2)
                nc.tensor.transpose(ps_t, xln[:, ts(k, 128)],
                                    ident[:1, :1])
                nc.scalar.copy(slotT[:, k, 1:2], ps_t)

            # ---- expert layer 1 (hidden-sharded); one [1,width] psum per
            # expert so everything stays on partition 0 ----
            wid = {"m": MH_SL, "l": EH_SL, "c": EH_SL, "s": EH_SL}
            srow = {"m": 0, "l": 1, "c": 2, "s": 3}
            wdram = {"m": mw1t_d, "l": ew1t_d["l"], "c": ew1t_d["c"],
                     "s": ew1t_d["s"]}
            ps_h1 = {e: psB.tile([1, wid[e]], F32, tag=f"h1{e}",
                                 name=f"ps_h1_{e}")
                     for e in ("m", "l", "c", "s")}
            for k in range(D // 128):
                for e in ("m", "l", "c", "s"):
                    we = pBs.tile([128, wid[e]], F32R, tag=f"w1{e}")
                    nc.sync.dma_start(out=we,
                                      in_=wdram[e][ts(k, 128), :].bitcast(F32R))
                    nc.tensor.matmul(ps_h1[e], slotT[:, k, srow[e]:srow[e] + 1],
                                     we, start=(k == 0),
                                     stop=(k == D // 128 - 1),
                                     skip_group_check=True)
            hrow = {}
            actf = {"m": AF.Gelu, "l": AF.Gelu, "c": AF.Silu, "s": AF.Tanh}
            b1nm = {"m": "mb1", "l": "lb1", "c": "cb1", "s": "sb1"}
            for e in ("m", "l", "c", "s"):
                hr = pB.tile([1, wid[e]], F32, tag=f"h1_{e}")
                nc.vector.tensor_add(hr, ps_h1[e], biases[b1nm[e]])
                hg = pB.tile([1, wid[e]], F32, tag=f"h1g_{e}")
                nc.scalar.activation(hg, hr, actf[e])
                hrow[e] = hg
            # transpose h rows for the K-contraction of layer 2
            hT = {}
            for e in ("m", "l", "c", "s"):
                nk = wid[e] // 128
                ht = pers.tile([128, nk, 1], F32R, tag=f"hT_{e}")
                for k in range(nk):
                    ps_t = psB.tile([128, 1], F32, tag="tpx", bufs=2)
                    nc.tensor.transpose(ps_t, hrow[e][:, ts(k, 128)],
                                        ident[:1, :1])
                    nc.scalar.copy(ht[:, k, :], ps_t)
                hT[e] = ht

        # layer-2 partials + AR2 (separate psum scope to stay in 8 banks)
        with tc.tile_pool(name="pB2", bufs=1) as pB2, \
             tc.tile_pool(name="pB2s", bufs=2) as pB2s, \
             tc.tile_pool(name="psB2", bufs=1, space="PSUM") as psB2:
            ar2_row = pB2.tile([1, H + 3 * D], F32)
            # math: h1g(512 slice of 4096) @ mw2t(512, H)
            ps_h2 = psB2.tile([1, H], F32, tag="yp")
            for k in range(MH_SL // 128):
                wc = pB2s.tile([128, H], F32R, tag="wm2")
                nc.sync.dma_start(out=wc, in_=mw2t_d[ts(k, 128), :].bitcast(F32R))
                for n in range(H // 512):
                    nc.tensor.matmul(ps_h2[:, ts(n, 512)], hT["m"][:, k, :],
                                     wc[:, ts(n, 512)], start=(k == 0),
                                     stop=(k == MH_SL // 128 - 1),
                                     skip_group_check=True)
            nc.scalar.copy(ar2_row[:, 0:H], ps_h2)
            for i, e in enumerate(("l", "c", "s")):
                ps_y = psB2.tile([1, D], F32, tag="yp")
                for k in range(EH_SL // 128):
                    wc = pB2s.tile([128, D], F32R, tag=f"w2{e}")
                    nc.sync.dma_start(out=wc,
                                      in_=ew2t_d[e][ts(k, 128), :].bitcast(F32R))
                    for n in range(D // 512):
                        nc.tensor.matmul(ps_y[:, ts(n, 512)], hT[e][:, k, :],
                                         wc[:, ts(n, 512)], start=(k == 0),
                                         stop=(k == EH_SL // 128 - 1),
                                         skip_group_check=True)
                nc.scalar.copy(ar2_row[:, H + i * D:H + (i + 1) * D],
                               ps_y)
            nc.gpsimd.dma_start(out=ar2_in[:, :], in_=ar2_row)

        nc.gpsimd.collective_compute(
            "AllReduce", mybir.AluOpType.add, replica_groups=RG,
            ins=[ar2_in[:, :]], outs=[ar2_out[:, :]])

        with tc.tile_pool(name="pB3", bufs=1) as pB3, \
             tc.tile_pool(name="pB3s", bufs=3) as pB3s, \
             tc.tile_pool(name="psB3", bufs=1, space="PSUM") as psB3:
            ar2_sb = pB3.tile([1, H + 3 * D], F32)
            nc.sync.dma_start(out=ar2_sb, in_=ar2_out[:, :])
            # math layer 2 finish: gelu(h2 + b2), then layer 3 with full w3
            h2 = pB3.tile([1, H], F32)
            nc.vector.tensor_add(h2, ar2_sb[:, 0:H], biases["mb2"])
            h2g = pB3.tile([1, H], F32)
            nc.scalar.activation(h2g, h2, AF.Gelu)
            h2T = pB3.tile([128, H // 128, 1], F32R)
            for k in range(H // 128):
                ps_t = psB3.tile([128, 1], F32, tag="tp4", bufs=2)
                nc.tensor.transpose(ps_t, h2g[:, ts(k, 128)], ident[:1, :1])
                nc.scalar.copy(h2T[:, k, :], ps_t)
            ps_y0 = psB3.tile([1, D], F32, tag="y0")
            for k in range(H // 128):
                wc = pB3s.tile([128, D], F32R, tag="wm3")
                nc.sync.dma_start(out=wc, in_=mw3t_d[ts(k, 128), :].bitcast(F32R))
                for n in range(D // 512):
                    nc.tensor.matmul(ps_y0[:, ts(n, 512)], h2T[:, k, :],
                                     wc[:, ts(n, 512)], start=(k == 0),
                                     stop=(k == H // 128 - 1),
                                     skip_group_check=True)
            # expert outputs: bias + clamp(-10, 10), assemble slot_out
            y = {}
            b2nm = {"l": "lb2", "c": "cb2", "s": "sb2"}
            y0 = pB3.tile([1, D], F32, tag="y_m")
            nc.vector.tensor_add(y0, ps_y0, biases["mb3"])
            y["m"] = y0
            for i, e in enumerate(("l", "c", "s")):
                ye = pB3.tile([1, D], F32, tag=f"y_{e}")
                nc.vector.tensor_add(
                    ye, ar2_sb[:, H + i * D:H + (i + 1) * D],
                    biases[b2nm[e]])
                y[e] = ye
            for e in ("m", "l", "c", "s"):
                nc.vector.tensor_scalar_min(y[e], y[e], 10.0)
                nc.vector.tensor_scalar_max(y[e], y[e], -10.0)
                nc.gpsimd.dma_start(out=slot_out[srow[e]:srow[e] + 1, :],
                                    in_=y[e])

            # ---- loss ----
            usage4 = pB3.tile([1, S], F32, tag="usage4")
            nc.vector.tensor_reduce(
                usage4,
                stats_tot[:, 0:16].rearrange("p (c s) -> p s c", c=4),
                axis=AX.X, op=OP.add)
            zsum = pB3.tile([1, 1], F32, tag="zsum")
            nc.vector.tensor_reduce(zsum, stats_tot[:, 16:20], axis=AX.X,
                                    op=OP.add)
            usc = pB3.tile([1, S], F32, tag="usc")
            nc.vector.tensor_scalar(usc, usage4, 1.0 / T, 1.0 / E_EXP,
                                    op0=OP.mult, op1=OP.subtract)
            uss = pB3.tile([1, 1], F32, tag="uss")
            usq = pB3.tile([1, S], F32, tag="usq")
            nc.scalar.activation(usq, usc, AF.Square, accum_out=uss)
            lsum = pB3.tile([1, 1], F32, tag="lsum")
            nc.scalar.activation(lsum, uss, AF.Copy, scale=float(E_EXP))
            t2 = pB3.tile([1, 1], F32, tag="t2")
            nc.scalar.activation(t2, zsum, AF.Copy, scale=Z_W / T)
            nc.vector.tensor_add(lsum, lsum, t2)
            t3 = pB3.tile([1, 1], F32, tag="t3")
            nc.scalar.activation(t3, ortho_ss, AF.Copy,
                                 scale=ORTHO_W / (S * (S - 1)))
            nc.vector.tensor_add(lsum, lsum, t3)
            nc.gpsimd.dma_start(out=loss_d[:, :], in_=lsum)

        bctx.close()

        # =====================  PHASE C  =====================
        with tc.tile_pool(name="pCw", bufs=1) as pCw, \
             tc.tile_pool(name="pCt", bufs=2) as pCt, \
             tc.tile_pool(name="psC", bufs=1, space="PSUM") as psC:
            w1t_sb = pCw.tile([128, D // 128, H], F32R)
            for k in range(D // 128):
                nc.sync.dma_start(out=w1t_sb[:, k, :],
                                  in_=w1t_d[ts(k, 128), :].bitcast(F32R))

            NH = H // 128      # 16 hidden blocks
            ND = D // 128      # 8 output blocks
            for tt in range(NTC):
                xt2 = pCt.tile([128, ND, TC], F32R, tag="xt2")
                nc.sync.dma_start(out=xt2, in_=xt_d[:, ts(tt, TC)].rearrange(
                    "(k p) c -> p k c", p=128).bitcast(F32R))
                cT = pCt.tile([S, TC], F32R, tag="cT")
                nc.sync.dma_start(out=cT,
                                  in_=combT_d[:, ts(tt, TC)].bitcast(F32R))
                hg = pCt.tile([128, NH, TC], F32R, tag="hg", bufs=1)
                for m in range(NH):
                    ph = psC.tile([128, TC], F32, tag="ph", bufs=3)
                    for k in range(ND):
                        nc.tensor.matmul(ph, w1t_sb[:, k, ts(m, 128)],
                                         xt2[:, k, :], start=(k == 0),
                                         stop=(k == ND - 1),
                                         skip_group_check=True)
                    nc.scalar.activation(hg[:, m, :], ph, AF.Gelu,
                                         bias=shb1[:, m:m + 1])
                ost = pCt.tile([128, ND, TC], F32, tag="ost", bufs=1)
                for mo in range(ND):
                    po = psC.tile([128, TC], F32, tag="po", bufs=3)
                    for k2 in range(NH):
                        nc.tensor.matmul(po, w2t_sb[:, k2, ts(mo, 128)],
                                         hg[:, k2, :], start=(k2 == 0),
                                         stop=False, skip_group_check=True)
                    nc.tensor.matmul(po, slot_out[:, ts(mo, 128)], cT,
                                     start=False, stop=True,
                                     skip_group_check=True)
                    nc.scalar.activation(ost[:, mo, :], po, AF.Identity,
                                         bias=shb2[:, mo:mo + 1])
                nc.gpsimd.dma_start(
                    out=outT_d[:, ts(tt, TC)].rearrange(
                        "(b p) c -> p b c", p=128),
                    in_=ost)

    nc.finalize()
    return nc


_PROGRAM = None


def _get_program():
    global _PROGRAM
    if _PROGRAM is None:
        _PROGRAM = _build_program()
    return _PROGRAM


def kernel(**inp):
    from concourse.bass_utils import run_bass_kernel_spmd

    x = np.asarray(inp["x"], dtype=np.float32)
    mw1 = np.asarray(inp["m_w1"], np.float32)
    mw2 = np.asarray(inp["m_w2"], np.float32)
    mw3 = np.asarray(inp["m_w3"], np.float32)

    xs = x.reshape(NCORES, TL, D)
    ident = np.eye(128, dtype=np.float32)
    bias_exp = np.repeat(np.asarray(inp["expert_bias"], np.float32), SPE)

    common = {
        "rwt": np.ascontiguousarray(np.asarray(inp["router_w"], np.float32).T),
        "rw": np.ascontiguousarray(np.asarray(inp["router_w"], np.float32)),
        "slot_bias": bias_exp.reshape(S, 1),
        "ident": ident,
        "w1t": np.ascontiguousarray(np.asarray(inp["sh_w1"], np.float32).T),
        "w2t": np.ascontiguousarray(np.asarray(inp["sh_w2"], np.float32).T),
        "shb1": np.ascontiguousarray(
            np.asarray(inp["sh_b1"], np.float32).reshape(H // 128, 128).T),
        "shb2": np.ascontiguousarray(
            np.asarray(inp["sh_b2"], np.float32).reshape(D // 128, 128).T),
        "mb2": np.asarray(inp["m_b2"], np.float32).reshape(1, H),
        "mw3t": np.ascontiguousarray(mw3.T),
        "mb3": np.asarray(inp["m_b3"], np.float32).reshape(1, D),
        "lb2": np.asarray(inp["l_b2"], np.float32).reshape(1, D),
        "cb2": np.asarray(inp["c_b2"], np.float32).reshape(1, D),
        "sb2": np.asarray(inp["s_b2"], np.float32).reshape(1, D),
        "lg": np.asarray(inp["l_g"], np.float32).reshape(1, D),
        "lbe": np.asarray(inp["l_be"], np.float32).reshape(1, D),
    }
    ew1 = {"l": inp["l_w1"], "c": inp["c_w1"], "s": inp["s_w1"]}
    ew2 = {"l": inp["l_w2"], "c": inp["c_w2"], "s": inp["s_w2"]}
    eb1 = {"l": inp["l_b1"], "c": inp["c_b1"], "s": inp["s_b1"]}

    in_maps = []
    for c in range(NCORES):
        m = dict(common)
        m["xm"] = np.ascontiguousarray(xs[c])
        m["xt"] = np.ascontiguousarray(xs[c].T)
        sl_m = slice(c * MH_SL, (c + 1) * MH_SL)
        sl_e = slice(c * EH_SL, (c + 1) * EH_SL)
        m["mw1t"] = np.ascontiguousarray(mw1[sl_m, :].T)
        m["mb1"] = np.asarray(inp["m_b1"], np.float32)[sl_m].reshape(1, MH_SL)
        m["mw2t"] = np.ascontiguousarray(mw2[:, sl_m].T)
        for e in ("l", "c", "s"):
            m[f"{e}w1t"] = np.ascontiguousarray(
                np.asarray(ew1[e], np.float32)[sl_e, :].T)
            m[f"{e}b1"] = np.asarray(eb1[e], np.float32)[sl_e].reshape(1, EH_SL)
            m[f"{e}w2t"] = np.ascontiguousarray(
                np.asarray(ew2[e], np.float32)[:, sl_e].T)
        in_maps.append(m)

    nc = _get_program()
    res = run_bass_kernel_spmd(nc, in_maps, core_ids=list(range(NCORES)))
    global LAST_EXEC_NS, LAST_RESULT
    LAST_EXEC_NS = res.exec_time_ns
    LAST_RESULT = res
    out = np.concatenate(
        [np.ascontiguousarray(res.results[c]["outT"].T) for c in range(NCORES)],
        axis=0)
    loss = np.float32(res.results[0]["loss"][0, 0])
    return out, loss


LAST_EXEC_NS = None
LAST_RESULT = None


# revision 22
# speedup vs baseline: 1.0435x; 1.0435x over previous
"""Soft-MoE (4 heterogeneous experts + shared expert) Trainium2 kernel.

Strategy (8 NeuronCores, data-parallel over tokens):
  - x sharded along T: each core gets Tl = 4096 tokens, in BOTH layouts
    (token-major for the dispatch contraction, D-major for everything else).
  - Phase A (per core): router logits, E = exp(logits), per-token combine,
    partial slot inputs P_l = E^T @ x_l, partial colsum/usage/z-loss stats.
  - AllReduce #1: P (4x1024) + stats row.
  - Phase B: slot_in = P/colsum; heterogeneous expert FFNs evaluated as
    GEMVs with hidden dims column-sharded across the 8 cores (uniform SPMD
    program, per-core weight slices fed as inputs); AllReduce #2 merges the
    second-layer partial sums; math expert layer 3 uses a replicated w3.
    Loss (router + z + ortho) computed on-device.
  - Phase C (per core): shared expert gelu(x@w1)@w2 with the slot-combine
    matmul accumulated into the same PSUM group, + per-D bias; output is
    written transposed (outT [D, Tl]) and re-assembled on the host.

All matmuls run in float32r (TF32-like, ~1.5e-4 rel err, full PE rate).
"""

import numpy as np

T, D, H = 32768, 1024, 2048
E_EXP, SPE = 4, 1
S = E_EXP * SPE
TAU = 1.0
Z_W, ORTHO_W = 0.01, 0.01
LN_EPS = 1e-5

NCORES = 8
TL = T // NCORES          # 4096 tokens per core
TA = 512                  # phase A token tile
NTA = TL // TA            # 8
TC = 256                  # phase C token tile
NTC = TL // TC            # 16
MH_SL = (2 * H) // NCORES  # 512  math hidden slice per core
EH_SL = H // NCORES        # 256  l/c/s hidden slice per core


def _build_program():
    import concourse.tile as tile
    from concourse import bacc, mybir
    from contextlib import ExitStack

    F32 = mybir.dt.float32
    F32R = mybir.dt.float32r
    AF = mybir.ActivationFunctionType
    OP = mybir.AluOpType
    AX = mybir.AxisListType

    def ts(i, n):
        return slice(i * n, (i + 1) * n)

    nc = bacc.Bacc("TRN2", target_bir_lowering=False, debug=False,
                   num_devices=NCORES)

    # ---- DRAM I/O ----
    xt_d = nc.dram_tensor("xt", [D, TL], F32, kind="ExternalInput")
    xm_d = nc.dram_tensor("xm", [TL, D], F32, kind="ExternalInput")
    rwt_d = nc.dram_tensor("rwt", [D, S], F32, kind="ExternalInput")
    rw_d = nc.dram_tensor("rw", [S, D], F32, kind="ExternalInput")
    sbias_d = nc.dram_tensor("slot_bias", [S, 1], F32, kind="ExternalInput")
    id_d = nc.dram_tensor("ident", [128, 128], F32, kind="ExternalInput")
    w1t_d = nc.dram_tensor("w1t", [D, H], F32, kind="ExternalInput")
    w2t_d = nc.dram_tensor("w2t", [H, D], F32, kind="ExternalInput")
    shb1_d = nc.dram_tensor("shb1", [128, H // 128], F32, kind="ExternalInput")
    shb2_d = nc.dram_tensor("shb2", [128, D // 128], F32, kind="ExternalInput")
    mw1t_d = nc.dram_tensor("mw1t", [D, MH_SL], F32, kind="ExternalInput")
    mb1_d = nc.dram_tensor("mb1", [1, MH_SL], F32, kind="ExternalInput")
    mw2t_d = nc.dram_tensor("mw2t", [MH_SL, H], F32, kind="ExternalInput")
    mb2_d = nc.dram_tensor("mb2", [1, H], F32, kind="ExternalInput")
    mw3t_d = nc.dram_tensor("mw3t", [H, D], F32, kind="ExternalInput")
    mb3_d = nc.dram_tensor("mb3", [1, D], F32, kind="ExternalInput")
    ew1t_d, eb1_d, ew2t_d, eb2_d = {}, {}, {}, {}
    for e in ("l", "c", "s"):
        ew1t_d[e] = nc.dram_tensor(f"{e}w1t", [D, EH_SL], F32, kind="ExternalInput")
        eb1_d[e] = nc.dram_tensor(f"{e}b1", [1, EH_SL], F32, kind="ExternalInput")
        ew2t_d[e] = nc.dram_tensor(f"{e}w2t", [EH_SL, D], F32, kind="ExternalInput")
        eb2_d[e] = nc.dram_tensor(f"{e}b2", [1, D], F32, kind="ExternalInput")
    lg_d = nc.dram_tensor("lg", [1, D], F32, kind="ExternalInput")
    lbe_d = nc.dram_tensor("lbe", [1, D], F32, kind="ExternalInput")

    outT_d = nc.dram_tensor("outT", [D, TL], F32, kind="ExternalOutput")
    loss_d = nc.dram_tensor("loss", [1, 1], F32, kind="ExternalOutput")

    combT_d = nc.dram_tensor("combT_scratch", [S, TL], F32)
    ar1_in = nc.dram_tensor("ar1_in", [5, D], F32)
    ar1_out = nc.dram_tensor("ar1_out", [5, D], F32, addr_space="Shared")
    ar2_in = nc.dram_tensor("ar2_in", [1, H + 3 * D], F32)
    ar2_out = nc.dram_tensor("ar2_out", [1, H + 3 * D], F32,
                             addr_space="Shared")
    RG = [list(range(NCORES))]

    with tile.TileContext(nc) as tc, ExitStack() as octx:
        pers = octx.enter_context(tc.tile_pool(name="pers", bufs=1))

        # resident small tensors
        rwt_sb = pers.tile([128, D // 128, S], F32R)
        nc.sync.dma_start(out=rwt_sb, in_=rwt_d[:, :].rearrange(
            "(k p) s -> p k s", p=128).bitcast(F32R))
        ident = pers.tile([128, 128], F32)
        nc.sync.dma_start(out=ident, in_=id_d[:, :])
        sbias = pers.tile([S, 1], F32)
        nc.sync.dma_start(out=sbias, in_=sbias_d[:, :])
        shb1 = pers.tile([128, H // 128], F32)
        nc.sync.dma_start(out=shb1, in_=shb1_d[:, :])
        shb2 = pers.tile([128, D // 128], F32)
        nc.sync.dma_start(out=shb2, in_=shb2_d[:, :])
        cs_acc = pers.tile([S, NTA], F32)      # exp colsum partials per tile
        slot_out = pers.tile([S, D], F32R)     # final expert outputs
        ortho_ss = pers.tile([1, 1], F32)
        ones128 = nc.const_aps.tensor(1.0, (128, 1))
        ones4 = nc.const_aps.tensor(1.0, (4, 1))

        # =====================  PHASE A  =====================
        with tc.tile_pool(name="pA", bufs=2) as pA, \
             tc.tile_pool(name="psA", bufs=1, space="PSUM") as psA:
            psum_P = psA.tile([S, D], F32)        # slot-input partials
            psum_zu = psA.tile([1, 20], F32)      # usage(16) + zsq(4)

            for t in range(NTA):
                xt_t = pA.tile([128, D // 128, TA], F32R, tag="xt")
                nc.sync.dma_start(out=xt_t, in_=xt_d[:, ts(t, TA)].rearrange(
                    "(k p) c -> p k c", p=128).bitcast(F32R))
                xm_t = pA.tile([128, TA // 128, D], F32R, tag="xm")
                nc.sync.dma_start(out=xm_t, in_=xm_d[ts(t, TA), :].rearrange(
                    "(c p) d -> p c d", p=128).bitcast(F32R))

                # logits^T [S, TA] (slot-major)
                ps_lt = psA.tile([S, TA], F32, tag="lt")
                for k in range(D // 128):
                    nc.tensor.matmul(ps_lt, rwt_sb[:, k, :], xt_t[:, k, :],
                                     start=(k == 0), stop=(k == D // 128 - 1))
                # E^T = exp(logits/TAU + bias); accum -> colsum partial
                eT = pA.tile([S, TA], F32, tag="eT")
                nc.scalar.activation(eT, ps_lt, AF.Exp, bias=sbias,
                                     scale=1.0 / TAU,
                                     accum_out=cs_acc[:, t:t + 1])

                e_tok = pA.tile([128, TA // 128, S], F32R, tag="etok")
                rs = pA.tile([128, TA // 128], F32, tag="rs")
                stats = pA.tile([128, 20], F32, tag="stats")
                combT_sb = pA.tile([S, TA], F32R, tag="cTs")
                for cch in range(TA // 128):
                    ps_e = psA.tile([128, S], F32, tag="pe_t", bufs=2)
                    nc.tensor.transpose(ps_e, eT[:, ts(cch, 128)],
                                        ident[:S, :S])
                    nc.scalar.copy(e_tok[:, cch, :], ps_e)
                    nc.vector.tensor_reduce(rs[:, cch:cch + 1], ps_e,
                                            axis=AX.X, op=OP.add)
                    rr = pA.tile([128, 1], F32, tag="rr", bufs=3)
                    nc.vector.reciprocal(rr, rs[:, cch:cch + 1])
                    # combine chunk
                    nc.vector.tensor_scalar_mul(
                        stats[:, cch * S:(cch + 1) * S], ps_e, rr)
                    # z-term chunk: min(log(rowsum),10)^2
                    zl = pA.tile([128, 1], F32, tag="zl", bufs=3)
                    nc.scalar.activation(zl, rs[:, cch:cch + 1], AF.Ln)
                    nc.vector.tensor_scalar_min(zl, zl, 10.0)
                    nc.scalar.activation(stats[:, 16 + cch:17 + cch], zl,
                                         AF.Square)
                    # combine^T for phase C
                    ps_ct = psA.tile([S, 128], F32, tag="ct", bufs=2)
                    nc.tensor.transpose(ps_ct, stats[:, cch * S:(cch + 1) * S],
                                        ident[:128, :128])
                    nc.scalar.copy(combT_sb[:, ts(cch, 128)], ps_ct)
                nc.gpsimd.dma_start(out=combT_d[:, ts(t, TA)].bitcast(F32R),
                                    in_=combT_sb)
                # stats reduction over tokens (partition sum via PE)
                nc.tensor.matmul(psum_zu, ones128, stats, start=(t == 0),
                                 stop=(t == NTA - 1), skip_group_check=True)
                # P partial: E^T @ x
                for cch in range(TA // 128):
                    for h in range(2):
                        nc.tensor.matmul(
                            psum_P[:, ts(h, 512)], e_tok[:, cch, :],
                            xm_t[:, cch, ts(h, 512)],
                            start=(t == 0 and cch == 0),
                            stop=(t == NTA - 1 and cch == TA // 128 - 1),
                            skip_group_check=True)

            # ---- ortho loss (router weight only) ----
            rw_sb = pA.tile([S, D], F32, tag="rw_sb")
            nc.sync.dma_start(out=rw_sb, in_=rw_d[:, :])
            scr = pA.tile([S, D], F32, tag="scr")
            nrm = pA.tile([S, 1], F32, tag="nrm")
            nc.scalar.activation(scr, rw_sb, AF.Square, accum_out=nrm)
            nc.scalar.activation(nrm, nrm, AF.Sqrt)
            nc.vector.tensor_scalar_add(nrm, nrm, 1e-8)
            rnrm = pA.tile([S, 1], F32, tag="rnrm")
            nc.vector.reciprocal(rnrm, nrm)
            wn = pA.tile([S, D], F32, tag="wn")
            nc.scalar.activation(wn, rw_sb, AF.Copy, scale=rnrm)
            wnT = pA.tile([128, D // 128, S], F32R, tag="wnT")
            for k in range(D // 128):
                ps_w = psA.tile([128, S], F32, tag="pe_t", bufs=2)
                nc.tensor.transpose(ps_w, wn[:, ts(k, 128)], ident[:S, :S])
                nc.scalar.copy(wnT[:, k, :], ps_w)
            ps_g = psA.tile([S, S], F32, tag="lt")
            for k in range(D // 128):
                nc.tensor.matmul(ps_g, wnT[:, k, :], wnT[:, k, :],
                                 start=(k == 0), stop=(k == D // 128 - 1))
            gram = pA.tile([S, S], F32, tag="gram_sb")
            nc.scalar.copy(gram, ps_g)
            nc.vector.tensor_sub(gram, gram, ident[:S, :S])
            osq = pA.tile([S, S], F32, tag="osq")
            orow = pA.tile([S, 1], F32, tag="orow")
            nc.scalar.activation(osq, gram, AF.Square, accum_out=orow)
            ps_o = psA.tile([1, 1], F32, tag="ct", bufs=2)
            nc.tensor.matmul(ps_o, ones4, orow, start=True, stop=True)
            nc.scalar.copy(ortho_ss, ps_o)

            # ---- pack + AllReduce #1 ----
            P_sb = pA.tile([S, D], F32, tag="P_sb")
            nc.scalar.copy(P_sb, psum_P)
            stats_row = pA.tile([1, D], F32, tag="stats_row")
            nc.vector.memset(stats_row, 0.0)
            nc.scalar.copy(stats_row[:, 0:20], psum_zu)
            colsum = pA.tile([S, 1], F32, tag="colsum")
            nc.vector.tensor_reduce(colsum, cs_acc, axis=AX.X, op=OP.add)
            ps_cs = psA.tile([1, S], F32, tag="ct", bufs=2)
            nc.tensor.transpose(ps_cs, colsum, ident[:S, :S])
            nc.scalar.copy(stats_row[:, 20:24], ps_cs)
            nc.gpsimd.dma_start(out=ar1_in[0:S, :], in_=P_sb)
            nc.gpsimd.dma_start(out=ar1_in[S:S + 1, :], in_=stats_row)

        nc.gpsimd.collective_compute(
            "AllReduce", mybir.AluOpType.add,
            replica_groups=RG, ins=[ar1_in[:, :]], outs=[ar1_out[:, :]])

        # =====================  PHASE B  =====================
        bctx = ExitStack()
        pBb = bctx.enter_context(tc.tile_pool(name="pBb", bufs=1))
        biases = {}
        for nm, dt_ in (("mb1", mb1_d), ("mb2", mb2_d), ("mb3", mb3_d),
                        ("lb1", eb1_d["l"]), ("lb2", eb2_d["l"]),
                        ("cb1", eb1_d["c"]), ("cb2", eb2_d["c"]),
                        ("sb1", eb1_d["s"]), ("sb2", eb2_d["s"]),
                        ("lg", lg_d), ("lbe", lbe_d)):
            t_ = pBb.tile(list(dt_.shape), F32, tag=f"bias_{nm}")
            nc.sync.dma_start(out=t_, in_=dt_[:, :])
            biases[nm] = t_
        with tc.tile_pool(name="pB", bufs=1) as pB, \
             tc.tile_pool(name="pBs", bufs=3) as pBs, \
             tc.tile_pool(name="psB", bufs=1, space="PSUM") as psB:
            P_tot = pB.tile([S, D], F32)
            nc.sync.dma_start(out=P_tot, in_=ar1_out[0:S, :])
            stats_tot = pers.tile([1, D], F32)
            nc.sync.dma_start(out=stats_tot, in_=ar1_out[S:S + 1, :])

            # slot_in = P / colsum
            ps_cs4 = psB.tile([S, 1], F32, tag="tpx", bufs=2)
            nc.tensor.transpose(ps_cs4, stats_tot[:, 20:24], ident[:1, :1])
            cs4 = pB.tile([S, 1], F32)
            nc.scalar.copy(cs4, ps_cs4)
            rc4 = pB.tile([S, 1], F32)
            nc.vector.reciprocal(rc4, cs4)
            slot_in = pB.tile([S, D], F32)
            nc.scalar.activation(slot_in, P_tot, AF.Copy, scale=rc4)
            slotT = pB.tile([128, D // 128, S], F32R)
            for k in range(D // 128):
                ps_t = psB.tile([128, S], F32, tag="tpx", bufs=2)
                nc.tensor.transpose(ps_t, slot_in[:, ts(k, 128)],
                                    ident[:S, :S])
                nc.scalar.copy(slotT[:, k, :], ps_t)

            # LayerNorm on language slot -> replace slotT[:, :, 1]
            xl_raw = pB.tile([1, D], F32)
            nc.gpsimd.dma_start(out=xl_raw, in_=P_tot[1:2, :])
            rc1 = pB.tile([1, 1], F32)
            nc.gpsimd.dma_start(out=rc1, in_=rc4[1:2, :])
            xl = pB.tile([1, D], F32)
            nc.scalar.activation(xl, xl_raw, AF.Copy, scale=rc1)
            bst = pB.tile([1, 2, 6], F32)
            for h in range(2):
                nc.vector.bn_stats(bst[:, h, :], xl[:, ts(h, 512)])
            mv = pB.tile([1, 2], F32)
            nc.vector.bn_aggr(mv, bst)
            eps_t = pB.tile([1, 1], F32)
            nc.vector.memset(eps_t, LN_EPS)
            sd = pB.tile([1, 1], F32)
            nc.scalar.activation(sd, mv[:, 1:2], AF.Sqrt, bias=eps_t)
            rstd = pB.tile([1, 1], F32)
            nc.vector.reciprocal(rstd, sd)
            xln = pB.tile([1, D], F32)
            nc.vector.tensor_scalar(xln, xl, mv[:, 0:1], rstd,
                                    op0=OP.subtract, op1=OP.mult)
            nc.vector.tensor_mul(xln, xln, biases["lg"])
            nc.vector.tensor_add(xln, xln, biases["lbe"])
            for k in range(D // 128):
                ps_t = psB.tile([128, 1], F32, tag="tpx", bufs=2)
                nc.tensor.transpose(ps_t, xln[:, ts(k, 128)],
                                    ident[:1, :1])
                nc.scalar.copy(slotT[:, k, 1:2], ps_t)

            # ---- expert layer 1 (hidden-sharded); one [1,width] psum per
            # expert so everything stays on partition 0 ----
            wid = {"m": MH_SL, "l": EH_SL, "c": EH_SL, "s": EH_SL}
            srow = {"m": 0, "l": 1, "c": 2, "s": 3}
            wdram = {"m": mw1t_d, "l": ew1t_d["l"], "c": ew1t_d["c"],
                     "s": ew1t_d["s"]}
            ps_h1 = {e: psB.tile([1, wid[e]], F32, tag=f"h1{e}",
                                 name=f"ps_h1_{e}")
                     for e in ("m", "l", "c", "s")}
            for k in range(D // 128):
                for e in ("m", "l", "c", "s"):
                    we = pBs.tile([128, wid[e]], F32R, tag=f"w1{e}")
                    nc.sync.dma_start(out=we,
                                      in_=wdram[e][ts(k, 128), :].bitcast(F32R))
                    nc.tensor.matmul(ps_h1[e], slotT[:, k, srow[e]:srow[e] + 1],
                                     we, start=(k == 0),
                                     stop=(k == D // 128 - 1),
                                     skip_group_check=True)
            hrow = {}
            actf = {"m": AF.Gelu, "l": AF.Gelu, "c": AF.Silu, "s": AF.Tanh}
            b1nm = {"m": "mb1", "l": "lb1", "c": "cb1", "s": "sb1"}
            for e in ("m", "l", "c", "s"):
                hr = pB.tile([1, wid[e]], F32, tag=f"h1_{e}")
                nc.vector.tensor_add(hr, ps_h1[e], biases[b1nm[e]])
                hg = pB.tile([1, wid[e]], F32, tag=f"h1g_{e}")
                nc.scalar.activation(hg, hr, actf[e])
                hrow[e] = hg
            # transpose h rows for the K-contraction of layer 2
            hT = {}
            for e in ("m", "l", "c", "s"):
                nk = wid[e] // 128
                ht = pers.tile([128, nk, 1], F32R, tag=f"hT_{e}")
                for k in range(nk):
                    ps_t = psB.tile([128, 1], F32, tag="tpx", bufs=2)
                    nc.tensor.transpose(ps_t, hrow[e][:, ts(k, 128)],
                                        ident[:1, :1])
                    nc.scalar.copy(ht[:, k, :], ps_t)
                hT[e] = ht

        # layer-2 partials + AR2 (separate psum scope to stay in 8 banks)
        with tc.tile_pool(name="pB2", bufs=1) as pB2, \
             tc.tile_pool(name="pB2s", bufs=2) as pB2s, \
             tc.tile_pool(name="psB2", bufs=1, space="PSUM") as psB2:
            ar2_row = pB2.tile([1, H + 3 * D], F32)
            # math: h1g(512 slice of 4096) @ mw2t(512, H)
            ps_h2 = psB2.tile([1, H], F32, tag="yp")
            for k in range(MH_SL // 128):
                wc = pB2s.tile([128, H], F32R, tag="wm2")
                nc.sync.dma_start(out=wc, in_=mw2t_d[ts(k, 128), :].bitcast(F32R))
                for n in range(H // 512):
                    nc.tensor.matmul(ps_h2[:, ts(n, 512)], hT["m"][:, k, :],
                                     wc[:, ts(n, 512)], start=(k == 0),
                                     stop=(k == MH_SL // 128 - 1),
                                     skip_group_check=True)
            nc.scalar.copy(ar2_row[:, 0:H], ps_h2)
            for i, e in enumerate(("l", "c", "s")):
                ps_y = psB2.tile([1, D], F32, tag="yp")
                for k in range(EH_SL // 128):
                    wc = pB2s.tile([128, D], F32R, tag=f"w2{e}")
                    nc.sync.dma_start(out=wc,
                                      in_=ew2t_d[e][ts(k, 128), :].bitcast(F32R))
                    for n in range(D // 512):
                        nc.tensor.matmul(ps_y[:, ts(n, 512)], hT[e][:, k, :],
                                         wc[:, ts(n, 512)], start=(k == 0),
                                         stop=(k == EH_SL // 128 - 1),
                                         skip_group_check=True)
                nc.scalar.copy(ar2_row[:, H + i * D:H + (i + 1) * D],
                               ps_y)
            nc.gpsimd.dma_start(out=ar2_in[:, :], in_=ar2_row)

        nc.gpsimd.collective_compute(
            "AllReduce", mybir.AluOpType.add, replica_groups=RG,
            ins=[ar2_in[:, :]], outs=[ar2_out[:, :]])

        with tc.tile_pool(name="pB3", bufs=1) as pB3, \
             tc.tile_pool(name="pB3s", bufs=3) as pB3s, \
             tc.tile_pool(name="psB3", bufs=1, space="PSUM") as psB3:
            ar2_sb = pB3.tile([1, H + 3 * D], F32)
            nc.sync.dma_start(out=ar2_sb, in_=ar2_out[:, :])
            # math layer 2 finish: gelu(h2 + b2), then layer 3 with full w3
            h2 = pB3.tile([1, H], F32)
            nc.vector.tensor_add(h2, ar2_sb[:, 0:H], biases["mb2"])
            h2g = pB3.tile([1, H], F32)
            nc.scalar.activation(h2g, h2, AF.Gelu)
            h2T = pB3.tile([128, H // 128, 1], F32R)
            for k in range(H // 128):
                ps_t = psB3.tile([128, 1], F32, tag="tp4", bufs=2)
                nc.tensor.transpose(ps_t, h2g[:, ts(k, 128)], ident[:1, :1])
                nc.scalar.copy(h2T[:, k, :], ps_t)
            ps_y0 = psB3.tile([1, D], F32, tag="y0")
            for k in range(H // 128):
                wc = pB3s.tile([128, D], F32R, tag="wm3")
                nc.sync.dma_start(out=wc, in_=mw3t_d[ts(k, 128), :].bitcast(F32R))
                for n in range(D // 512):
                    nc.tensor.matmul(ps_y0[:, ts(n, 512)], h2T[:, k, :],
                                     wc[:, ts(n, 512)], start=(k == 0),
                                     stop=(k == H // 128 - 1),
                                     skip_group_check=True)
            # expert outputs: bias + clamp(-10, 10), assemble slot_out
            y = {}
            b2nm = {"l": "lb2", "c": "cb2", "s": "sb2"}
            y0 = pB3.tile([1, D], F32, tag="y_m")
            nc.vector.tensor_add(y0, ps_y0, biases["mb3"])
            y["m"] = y0
            for i, e in enumerate(("l", "c", "s")):
                ye = pB3.tile([1, D], F32, tag=f"y_{e}")
                nc.vector.tensor_add(
                    ye, ar2_sb[:, H + i * D:H + (i + 1) * D],
                    biases[b2nm[e]])
                y[e] = ye
            for e in ("m", "l", "c", "s"):
                nc.vector.tensor_scalar_min(y[e], y[e], 10.0)
                nc.vector.tensor_scalar_max(y[e], y[e], -10.0)
                nc.gpsimd.dma_start(out=slot_out[srow[e]:srow[e] + 1, :],
                                    in_=y[e])

            # ---- loss ----
            usage4 = pB3.tile([1, S], F32, tag="usage4")
            nc.vector.tensor_reduce(
                usage4,
                stats_tot[:, 0:16].rearrange("p (c s) -> p s c", c=4),
                axis=AX.X, op=OP.add)
            zsum = pB3.tile([1, 1], F32, tag="zsum")
            nc.vector.tensor_reduce(zsum, stats_tot[:, 16:20], axis=AX.X,
                                    op=OP.add)
            usc = pB3.tile([1, S], F32, tag="usc")
            nc.vector.tensor_scalar(usc, usage4, 1.0 / T, 1.0 / E_EXP,
                                    op0=OP.mult, op1=OP.subtract)
            uss = pB3.tile([1, 1], F32, tag="uss")
            usq = pB3.tile([1, S], F32, tag="usq")
            nc.scalar.activation(usq, usc, AF.Square, accum_out=uss)
            lsum = pB3.tile([1, 1], F32, tag="lsum")
            nc.scalar.activation(lsum, uss, AF.Copy, scale=float(E_EXP))
            t2 = pB3.tile([1, 1], F32, tag="t2")
            nc.scalar.activation(t2, zsum, AF.Copy, scale=Z_W / T)
            nc.vector.tensor_add(lsum, lsum, t2)
            t3 = pB3.tile([1, 1], F32, tag="t3")
            nc.scalar.activation(t3, ortho_ss, AF.Copy,
                                 scale=ORTHO_W / (S * (S - 1)))
            nc.vector.tensor_add(lsum, lsum, t3)
            nc.gpsimd.dma_start(out=loss_d[:, :], in_=lsum)

        bctx.close()

        # =====================  PHASE C  =====================
        with tc.tile_pool(name="pCw", bufs=1) as pCw, \
             tc.tile_pool(name="pCt", bufs=2) as pCt, \
             tc.tile_pool(name="psC", bufs=1, space="PSUM") as psC:
            w1t_sb = pCw.tile([128, D // 128, H], F32R)
            for k in range(D // 128):
                nc.sync.dma_start(out=w1t_sb[:, k, :],
                                  in_=w1t_d[ts(k, 128), :].bitcast(F32R))
            w2t_sb = pCw.tile([128, H // 128, D], F32R)
            for k in range(H // 128):
                nc.sync.dma_start(out=w2t_sb[:, k, :],
                                  in_=w2t_d[ts(k, 128), :].bitcast(F32R))

            NH = H // 128      # 16 hidden blocks
            ND = D // 128      # 8 output blocks
            for tt in range(NTC):
                xt2 = pCt.tile([128, ND, TC], F32R, tag="xt2")
                nc.sync.dma_start(out=xt2, in_=xt_d[:, ts(tt, TC)].rearrange(
                    "(k p) c -> p k c", p=128).bitcast(F32R))
                cT = pCt.tile([S, TC], F32R, tag="cT")
                nc.sync.dma_start(out=cT,
                                  in_=combT_d[:, ts(tt, TC)].bitcast(F32R))
                hg = pCt.tile([128, NH, TC], F32R, tag="hg", bufs=1)
                for m in range(NH):
                    ph = psC.tile([128, TC], F32, tag="ph", bufs=2)
                    for k in range(ND):
                        nc.tensor.matmul(ph, w1t_sb[:, k, ts(m, 128)],
                                         xt2[:, k, :], start=(k == 0),
                                         stop=(k == ND - 1),
                                         skip_group_check=True)
                    nc.scalar.activation(hg[:, m, :], ph, AF.Gelu,
                                         bias=shb1[:, m:m + 1])
                ost = pCt.tile([128, ND, TC], F32, tag="ost", bufs=1)
                for mo in range(ND):
                    po = psC.tile([128, TC], F32, tag="po", bufs=2)
                    for k2 in range(NH):
                        nc.tensor.matmul(po, w2t_sb[:, k2, ts(mo, 128)],
                                         hg[:, k2, :], start=(k2 == 0),
                                         stop=False, skip_group_check=True)
                    nc.tensor.matmul(po, slot_out[:, ts(mo, 128)], cT,
                                     start=False, stop=True,
                                     skip_group_check=True)
                    nc.scalar.activation(ost[:, mo, :], po, AF.Identity,
                                         bias=shb2[:, mo:mo + 1])
                nc.gpsimd.dma_start(
                    out=outT_d[:, ts(tt, TC)].rearrange(
                        "(b p) c -> p b c", p=128),
                    in_=ost)

    nc.finalize()
    return nc


_PROGRAM = None


def _get_program():
    global _PROGRAM
    if _PROGRAM is None:
        _PROGRAM = _build_program()
    return _PROGRAM


def kernel(**inp):
    from concourse.bass_utils import run_bass_kernel_spmd

    x = np.asarray(inp["x"], dtype=np.float32)
    mw1 = np.asarray(inp["m_w1"], np.float32)
    mw2 = np.asarray(inp["m_w2"], np.float32)
    mw3 = np.asarray(inp["m_w3"], np.float32)

    xs = x.reshape(NCORES, TL, D)
    ident = np.eye(128, dtype=np.float32)
    bias_exp = np.repeat(np.asarray(inp["expert_bias"], np.float32), SPE)

    common = {
        "rwt": np.ascontiguousarray(np.asarray(inp["router_w"], np.float32).T),
        "rw": np.ascontiguousarray(np.asarray(inp["router_w"], np.float32)),
        "slot_bias": bias_exp.reshape(S, 1),
        "ident": ident,
        "w1t": np.ascontiguousarray(np.asarray(inp["sh_w1"], np.float32).T),
        "w2t": np.ascontiguousarray(np.asarray(inp["sh_w2"], np.float32).T),
        "shb1": np.ascontiguousarray(
            np.asarray(inp["sh_b1"], np.float32).reshape(H // 128, 128).T),
        "shb2": np.ascontiguousarray(
            np.asarray(inp["sh_b2"], np.float32).reshape(D // 128, 128).T),
        "mb2": np.asarray(inp["m_b2"], np.float32).reshape(1, H),
        "mw3t": np.ascontiguousarray(mw3.T),
        "mb3": np.asarray(inp["m_b3"], np.float32).reshape(1, D),
        "lb2": np.asarray(inp["l_b2"], np.float32).reshape(1, D),
        "cb2": np.asarray(inp["c_b2"], np.float32).reshape(1, D),
        "sb2": np.asarray(inp["s_b2"], np.float32).reshape(1, D),
        "lg": np.asarray(inp["l_g"], np.float32).reshape(1, D),
        "lbe": np.asarray(inp["l_be"], np.float32).reshape(1, D),
    }
    ew1 = {"l": inp["l_w1"], "c": inp["c_w1"], "s": inp["s_w1"]}
    ew2 = {"l": inp["l_w2"], "c": inp["c_w2"], "s": inp["s_w2"]}
    eb1 = {"l": inp["l_b1"], "c": inp["c_b1"], "s": inp["s_b1"]}

    in_maps = []
    for c in range(NCORES):
        m = dict(common)
        m["xm"] = np.ascontiguousarray(xs[c])
        m["xt"] = np.ascontiguousarray(xs[c].T)
        sl_m = slice(c * MH_SL, (c + 1) * MH_SL)
        sl_e = slice(c * EH_SL, (c + 1) * EH_SL)
        m["mw1t"] = np.ascontiguousarray(mw1[sl_m, :].T)
        m["mb1"] = np.asarray(inp["m_b1"], np.float32)[sl_m].reshape(1, MH_SL)
        m["mw2t"] = np.ascontiguousarray(mw2[:, sl_m].T)
        for e in ("l", "c", "s"):
            m[f"{e}w1t"] = np.ascontiguousarray(
                np.asarray(ew1[e], np.float32)[sl_e, :].T)
            m[f"{e}b1"] = np.asarray(eb1[e], np.float32)[sl_e].reshape(1, EH_SL)
            m[f"{e}w2t"] = np.ascontiguousarray(
                np.asarray(ew2[e], np.float32)[:, sl_e].T)
        in_maps.append(m)

    nc = _get_program()
    res = run_bass_kernel_spmd(nc, in_maps, core_ids=list(range(NCORES)))
    global LAST_EXEC_NS, LAST_RESULT
    LAST_EXEC_NS = res.exec_time_ns
    LAST_RESULT = res
    out = np.concatenate(
        [np.ascontiguousarray(res.results[c]["outT"].T) for c in range(NCORES)],
        axis=0)
    loss = np.float32(res.results[0]["loss"][0, 0])
    return out, loss


LAST_EXEC_NS = None
LAST_RESULT = None
